# revision 1
# baseline (speedup 1.0000x reference)
"""Multi-head attention layer (B=4, L=2048, D=1024, H=16) on 8 TRN2 NeuronCores.

Sharding: core c handles batch b = c//2 and heads [8*(c%2), 8*(c%2)+8) —
batch-parallel x tensor-parallel over heads.  Host sums the two partial
outputs per batch and adds bv@Wo + bo (bk drops exactly by softmax shift
invariance).

Per-core dataflow (all matmul inputs bf16, fp32 accumulation):
  qT/kT = Wq/Wk_slice as stationary against xT  -> [512, 2048] (e on partitions)
  v     = x @ Wv_slice (+ones col per head)      -> [2048, 8*65]
  scores transposed: ST[s, l] chunks = kT_h stationary vs qT_h
  A = exp(ST/8) on ACT (fp32 in, bf16 out), [128, 1024] payloads
  V[l, 65] = A_chunk.T @ v_aug accumulation (ones col -> softmax denom in
  col 64); normalize via per-partition reciprocal+tensor_scalar (DVE);
  PE-transpose V pairs into VT; out_partial = VT.T @ Wo_slice.

Scheduling: blocks are pair-major (m outer, lt inner).  Only the pair-0
q/k projections run before the first block; the v and remaining q/k
projections stream into the early blocks as psum-tagged quanta so the
first exp starts ~40us in.  Each block emits the previous block's AV
sweep as one dense PE burst after score-group 3 (keeps ACT fed and the
PE HAM clock warm).  Score matmuls are issued h2-adjacent targeting the
two 64-partition PE row groups.
"""

import sys
from contextlib import ExitStack

for _p in ("/opt/trn_rl_repo", "/root/.axon_site/_ro/trn_rl_repo"):
    if _p not in sys.path:
        sys.path.append(_p)

import numpy as np
import ml_dtypes

import concourse.bass as bass
import concourse.mybir as mybir
import concourse.tile as tile
from concourse import bacc
from concourse.bass_utils import run_bass_kernel_spmd
from concourse.masks import make_identity

BF16 = mybir.dt.bfloat16
F32 = mybir.dt.float32
AF = mybir.ActivationFunctionType

B, L, D = 4, 2048, 1024
N_CORES = 8
DH = 512          # per-core head dims (8 heads x 64)
E = 64
SCALE = 0.125     # 1/sqrt(E)

KD = D // 128     # 8 contraction chunks for projections
NL = L // 512     # 4 l-tiles
NS = L // 128     # 16 s-chunks / l-subs


def build_attention_nc():
    nc = bacc.Bacc("TRN2", target_bir_lowering=False, debug=False)

    xT_d = nc.dram_tensor("xT", [D, L], BF16, kind="ExternalInput").ap()
    wq_d = nc.dram_tensor("wq", [D, DH], BF16, kind="ExternalInput").ap()
    wk_d = nc.dram_tensor("wk", [D, DH], BF16, kind="ExternalInput").ap()
    wv_d = nc.dram_tensor("wv", [D, DH], BF16, kind="ExternalInput").ap()
    wo_d = nc.dram_tensor("wo", [DH, D], BF16, kind="ExternalInput").ap()
    bq_d = nc.dram_tensor("bq", [DH, 1], F32, kind="ExternalInput").ap()
    out_d = nc.dram_tensor("out", [L, D], F32, kind="ExternalOutput").ap()

    with tile.TileContext(nc) as tc, ExitStack() as ctx:
        const_pool = ctx.enter_context(tc.tile_pool(name="const", bufs=1))
        w_pool = ctx.enter_context(tc.tile_pool(name="w", bufs=1))
        qk_pool = ctx.enter_context(tc.tile_pool(name="qk", bufs=1))
        v_pool = ctx.enter_context(tc.tile_pool(name="v", bufs=1))
        at_pool = ctx.enter_context(tc.tile_pool(name="at", bufs=14))
        vs_pool = ctx.enter_context(tc.tile_pool(name="vs", bufs=1))
        vt_pool = ctx.enter_context(tc.tile_pool(name="vt", bufs=8))
        rec_pool = ctx.enter_context(tc.tile_pool(name="rec", bufs=8))
        osb_pool = ctx.enter_context(tc.tile_pool(name="osb", bufs=2))

        st_ps = ctx.enter_context(tc.tile_pool(name="st_ps", bufs=1, space="PSUM"))
        av_ps = ctx.enter_context(tc.tile_pool(name="av_ps", bufs=1, space="PSUM"))
        tr_ps = ctx.enter_context(tc.tile_pool(name="tr_ps", bufs=1, space="PSUM"))
        out_ps = ctx.enter_context(tc.tile_pool(name="out_ps", bufs=1, space="PSUM"))
        phase1_ctx = ExitStack()
        xt_pool = phase1_ctx.enter_context(tc.tile_pool(name="xt", bufs=1))

        ident = const_pool.tile([128, 128], BF16, tag="ident", name="ident")
        make_identity(nc, ident[:])
        bq_sb = const_pool.tile([128, 4], F32, tag="bq", name="bq_sb")
        for m in range(4):
            nc.sync.dma_start(bq_sb[:, m : m + 1], bq_d[128 * m : 128 * m + 128, :])

        xt = []
        for i in range(KD):
            t = xt_pool.tile([128, L], BF16, tag=f"xt{i}", name=f"xt{i}")
            nc.sync.dma_start(t[:], xT_d[128 * i : 128 * i + 128, :])
            xt.append(t)
        wq, wk, wv = [], [], []
        for name, lst, dram in (("wq", wq, wq_d), ("wk", wk, wk_d), ("wv", wv, wv_d)):
            for i in range(KD):
                t = xt_pool.tile([128, DH], BF16, tag=f"{name}{i}", name=f"{name}{i}")
                nc.sync.dma_start(t[:], dram[128 * i : 128 * i + 128, :])
                lst.append(t)
        wo = []
        for p in range(DH // 128):
            t = w_pool.tile([128, D], BF16, tag=f"wo{p}", name=f"wo{p}")
            nc.sync.dma_start(t[:], wo_d[128 * p : 128 * p + 128, :])
            wo.append(t)

        qT = [qk_pool.tile([128, L], BF16, tag=f"qT{m}", name=f"qT{m}") for m in range(4)]
        kT = [qk_pool.tile([128, L], BF16, tag=f"kT{m}", name=f"kT{m}") for m in range(4)]

        def emit_qk_proj(m, which, n, tag):
            # one psum group (8 matmuls) of the q or k projection
            if tag in ("outp",):
                ps = out_ps.tile([128, 512], F32, tag=tag, name="proj")
            elif tag in ("tr",):
                ps = tr_ps.tile([128, 512], F32, tag=tag, name="proj")
            elif tag.startswith("av"):
                ps = av_ps.tile([128, 260], F32, tag=tag, name="proj")
            else:
                ps = st_ps.tile([128, 512], F32, tag=tag, name="proj")
            w_ = wq if which == "q" else wk
            for kd in range(KD):
                nc.tensor.matmul(
                    ps[:], w_[kd][:, 128 * m : 128 * m + 128],
                    xt[kd][:, 512 * n : 512 * n + 512],
                    start=(kd == 0), stop=(kd == KD - 1))
            if which == "q":
                nc.vector.tensor_scalar_add(
                    qT[m][:, 512 * n : 512 * n + 512], ps[:], bq_sb[:, m : m + 1])
            else:
                nc.vector.tensor_copy(kT[m][:, 512 * n : 512 * n + 512], ps[:])

        # ---- prologue: just qk pair 0 — everything else streams into the
        #      early blocks so the first exp starts as soon as possible ----
        rot = ["st0", "st1", "outp", "tr"]
        for n in range(NL):
            emit_qk_proj(0, "q", n, rot[n % 4])
        for n in range(NL):
            emit_qk_proj(0, "k", n, rot[(n + 2) % 4])

        v_aug = [None] * NS
        vrot = ["av0", "av1", "outp", "tr"]

        def emit_v_proj(s, tag):
            pool = av_ps if tag.startswith("av") else out_ps if tag == "outp" else tr_ps
            ps = pool.tile([128, 512], F32, tag=tag, name="proj")
            for kd in range(KD):
                nc.tensor.matmul(
                    ps[:], xt[kd][:, 128 * s : 128 * s + 128], wv[kd][:],
                    start=(kd == 0), stop=(kd == KD - 1))
            t = v_pool.tile([128, 520], BF16, tag=f"v{s}", name=f"vaug{s}")
            t3 = t[:].rearrange("p (h e) -> p h e", h=8)
            nc.vector.tensor_copy(t3[:, :, 0:64], ps[:].rearrange("p (h e) -> p h e", h=8))
            nc.vector.memset(t3[:, :, 64:65], 1.0)
            v_aug[s] = t

        # ---- attention blocks, pair-major ----
        vstage = [vs_pool.tile([128, DH], BF16, tag=f"vs{ls}", name=f"vs{ls}") for ls in range(NS)]

        def emit_block(m, lt, prev, v_quanta=False):
            ats = {0: [], 1: []}
            for g in range(8):
                stp = {}
                for h2 in range(2):
                    stp[h2] = st_ps.tile([128, 1024], F32, tag=f"st{h2}", name=f"st{h2}")
                for c2 in range(2):
                    s = 2 * g + c2
                    for h2 in range(2):
                        p0 = 64 * h2
                        nc.tensor.matmul(
                            stp[h2][:, 512 * c2 : 512 * c2 + 512],
                            kT[m][p0 : p0 + 64, 128 * s : 128 * s + 128],
                            qT[m][p0 : p0 + 64, 512 * lt : 512 * lt + 512],
                            start=True, stop=True)
                for h2 in range(2):
                    at = at_pool.tile([128, 1024], BF16, tag=f"at{h2}", name=f"at{h2}")
                    nc.scalar.activation(at[:], stp[h2][:], AF.Exp, scale=SCALE)
                    ats[h2].append(at)
                if g == 3 and prev is not None:
                    emit_av(*prev)
                if v_quanta:
                    for s2 in (2 * g, 2 * g + 1):
                        emit_v_proj(s2, vrot[s2 % 4])
            return ats

        def emit_av(m, lt, ats):
            # dense AV burst + normalize into vstage for block (m, lt)
            for h2 in range(2):
                h = 2 * m + h2
                avp = av_ps.tile([128, 260], F32, tag=f"av{h2}", name=f"av{h2}")
                for j in range(4):
                    for s in range(NS):
                        g, c2 = divmod(s, 2)
                        nc.tensor.matmul(
                            avp[:, 65 * j : 65 * j + 65],
                            ats[h2][g][:, 512 * c2 + 128 * j : 512 * c2 + 128 * j + 128],
                            v_aug[s][:, 65 * h : 65 * h + 65],
                            start=(s == 0), stop=(s == NS - 1))
                for j in range(4):
                    r = rec_pool.tile([128, 1], F32, tag="rec", name="rec")
                    nc.vector.reciprocal(r[:], avp[:, 65 * j + 64 : 65 * j + 65])
                    nc.vector.tensor_scalar_mul(
                        vstage[4 * lt + j][:, 64 * h : 64 * h + 64],
                        avp[:, 65 * j : 65 * j + 64], r[:])

        def emit_outproj(lt):
            for ls in range(4 * lt, 4 * lt + 4):
                vts = []
                for p in range(4):
                    tp = tr_ps.tile([128, 128], BF16, tag="tr", name="trp")
                    nc.tensor.transpose(tp[:], vstage[ls][:, 128 * p : 128 * p + 128], ident[:])
                    vt = vt_pool.tile([128, 128], BF16, tag="vt", name="vt")
                    nc.vector.tensor_copy(vt[:], tp[:])
                    vts.append(vt)
                osb = osb_pool.tile([128, D], F32, tag="osb", name="osb")
                for d2 in range(2):
                    op = out_ps.tile([128, 512], F32, tag="outp", name="outp")
                    for p in range(4):
                        nc.tensor.matmul(
                            op[:], vts[p][:], wo[p][:, 512 * d2 : 512 * d2 + 512],
                            start=(p == 0), stop=(p == 3))
                    nc.vector.tensor_copy(osb[:, 512 * d2 : 512 * d2 + 512], op[:])
                nc.sync.dma_start(out_d[128 * ls : 128 * ls + 128, :], osb[:])

        # remaining projections, two psum-groups per block while pair m runs
        proj_quanta = {m: [(m + 1, w, n) for w in ("q", "k") for n in range(NL)]
                       for m in range(3)}

        prev = None
        for m in range(4):
            for lt in range(NL):
                ats = emit_block(m, lt, prev, v_quanta=(m == 0 and lt == 0))
                if m < 3:
                    q = proj_quanta[m]
                    for qi in range(2):
                        if q:
                            pm, w, n = q.pop(0)
                            emit_qk_proj(pm, w, n, "outp" if qi == 0 else "tr")
                if m == 3 and lt > 0:
                    emit_outproj(lt - 1)
                prev = (m, lt, ats)
        emit_av(*prev)
        emit_outproj(3)
        phase1_ctx.close()

    nc.compile()
    return nc


_NC_CACHE = []


def _make_in_maps(inputs):
    x = np.asarray(inputs["x"], dtype=np.float32)
    Wq = np.asarray(inputs["Wq"], dtype=np.float32)
    Wk = np.asarray(inputs["Wk"], dtype=np.float32)
    Wv = np.asarray(inputs["Wv"], dtype=np.float32)
    Wo = np.asarray(inputs["Wo"], dtype=np.float32)
    bq = np.asarray(inputs["bq"], dtype=np.float32)
    bf = ml_dtypes.bfloat16
    in_maps = []
    for c in range(N_CORES):
        b, hh = divmod(c, 2)
        sl = slice(DH * hh, DH * hh + DH)
        in_maps.append({
            "xT": np.ascontiguousarray(x[b].T).astype(bf),
            "wq": np.ascontiguousarray(Wq[:, sl]).astype(bf),
            "wk": np.ascontiguousarray(Wk[:, sl]).astype(bf),
            "wv": np.ascontiguousarray(Wv[:, sl]).astype(bf),
            "wo": np.ascontiguousarray(Wo[sl, :]).astype(bf),
            "bq": np.ascontiguousarray(bq[sl]).reshape(DH, 1).astype(np.float32),
        })
    return in_maps


def kernel(x, Wq, bq, Wk, bk, Wv, bv, Wo, bo):
    x = np.asarray(x, dtype=np.float32)
    Wq = np.asarray(Wq, dtype=np.float32)
    Wk = np.asarray(Wk, dtype=np.float32)
    Wv = np.asarray(Wv, dtype=np.float32)
    Wo = np.asarray(Wo, dtype=np.float32)
    bq = np.asarray(bq, dtype=np.float32)
    bv = np.asarray(bv, dtype=np.float32)
    bo = np.asarray(bo, dtype=np.float32)

    if not _NC_CACHE:
        _NC_CACHE.append(build_attention_nc())
    nc = _NC_CACHE[0]

    in_maps = _make_in_maps(dict(x=x, Wq=Wq, bq=bq, Wk=Wk, Wv=Wv, Wo=Wo))

    res = run_bass_kernel_spmd(nc, in_maps, list(range(N_CORES)))
    parts = [res.results[c]["out"] for c in range(N_CORES)]
    out = np.stack([parts[2 * b] + parts[2 * b + 1] for b in range(B)])
    out += (bv @ Wo + bo)[None, None, :]
    return out.astype(np.float32)

